# revision 19
# baseline (speedup 1.0000x reference)
"""Trainium2 Bass kernel for nn_HadaMard: fused proj + 2xLayerNorm + outer product.

Reference computation (per batch b, one NeuronCore per batch):
  qf = q[b].reshape(C1, N)           # [1024, 1024]
  proj = Wp @ qf (+ bp)              # [256, 1024]
  qn = LN_over_d(proj) * g1 + b1     # LN over the 256-channel dim
  xn = LN_over_e(x[b]) * g2 + b2     # LN over the 32-channel dim
  out[d*32+e, n] = qn[d, n] * xn[e, n]   # [8192, 1024]

Layout/strategy:
  - Output is transferred in bf16 (rel-err ~6e-3 << 2e-2 gate) and upcast on
    host: halves the dominant HBM write traffic.
  - Outer-product tiles are e-major: tile (md, e) holds out rows
    (128*md+p)*32+e for p in [0,128). The qn factor is the bf16 qn tile
    itself (no broadcast); the xn factor is one row broadcast to all 128
    partitions.
  - Row broadcasts go through a DRAM scratch roundtrip: A = xn is written
    once (ready early, x-side only), then each xbe tile is a stride-0
    partition-broadcast DMA read. These land on the DMA queues
    (sync/scalar/gpsimd) during the otherwise-idle window while the q-side
    LN chain runs, instead of loading the busy compute engines.
  - Elementwise products run on DVE (bf16 2x mode) and Pool, DMAs on
    sync/scalar/gpsimd, assigned by a static least-loaded balancer.
  - The q side is processed in two 512-column chunks (PSUM bank granularity);
    early e's run per-chunk products to start output DMA sooner, later e's
    run full-width products.
"""

import numpy as np

_CACHE = {}

B, C1, H, W = 8, 1024, 32, 32
C2 = 32
Cp = 256
N = H * W  # 1024
CD = Cp * C2  # 8192
EPS = 1e-5

ESPLIT = 8  # e < ESPLIT: per-chunk products; else full-width
NWU = 6  # PE warm-up matmuls
NFILL = 4  # PE filler matmuls between proj chunks


def _build_nc(simple):
    import concourse.bacc as bacc
    import concourse.bass as bass
    import concourse.mybir as mybir
    import concourse.tile as tile

    F32 = mybir.dt.float32
    F32R = mybir.dt.float32r
    BF16 = mybir.dt.bfloat16
    SQRT = mybir.ActivationFunctionType.Sqrt
    COPY = mybir.ActivationFunctionType.Copy
    MULT = mybir.AluOpType.mult
    ADD = mybir.AluOpType.add

    nc = bacc.Bacc(None, target_bir_lowering=False)

    q_d = nc.dram_tensor("qb", [C1, N], BF16, kind="ExternalInput")
    w_d = nc.dram_tensor("wT", [C1, Cp], BF16, kind="ExternalInput")
    x_d = nc.dram_tensor("x", [C2, N], F32, kind="ExternalInput")
    bp_d = nc.dram_tensor("bpc", [128, 2], F32, kind="ExternalInput")
    g1_d = nc.dram_tensor("g1c", [128, 2], F32, kind="ExternalInput")
    b1_d = nc.dram_tensor("b1c", [128, 2], F32, kind="ExternalInput")
    g2_d = nc.dram_tensor("g2r", [C2, 1], F32, kind="ExternalInput")
    b2_d = nc.dram_tensor("b2r", [C2, 1], F32, kind="ExternalInput")
    abuf_d = nc.dram_tensor("abuf", [C2, N], BF16, kind="Internal")
    out_d = nc.dram_tensor("out", [CD, N], BF16, kind="ExternalOutput")

    # out view: row (md*128+p)*32+e  ->  [p, md, e, n]
    outv = out_d.rearrange("(md p e) n -> p md e n", md=2, p=128, e=C2)

    # ---- static least-loaded balancer (model-cost ns) ----
    clk = {"sync": 0.0, "scalar": 0.0, "gpsimd": 0.0, "vector": 0.0}

    def pick(cands, costs):
        e = min(cands, key=lambda x: clk[x])
        clk[e] += costs[e] if isinstance(costs, dict) else costs
        return e

    def charge(e, cost):
        clk[e] += cost

    DMA_ENGS = ["sync", "scalar", "gpsimd"]

    def dma_cost(bytes_per_part):
        return max(bytes_per_part * 0.3855, 500.0)

    def mul_costs(w):
        return {"vector": w * 1.0417 * 0.5 + 60.0, "gpsimd": w * 0.8333}

    CHUNKS = [(0, 512), (512, 1024)]

    with tile.TileContext(nc) as tc:
        with (
            tc.tile_pool(name="cst", bufs=1) as cst,
            tc.tile_pool(name="big", bufs=1) as big,
            tc.tile_pool(name="xbe", bufs=1) as xbp,
            tc.tile_pool(name="stg", bufs=7) as stg,
            tc.tile_pool(name="stc", bufs=4) as stc,
            tc.tile_pool(name="wrk", bufs=2) as wrk,
            tc.tile_pool(name="ps", bufs=4, space=bass.MemorySpace.PSUM) as ps,
            tc.tile_pool(name="wups", bufs=1, space=bass.MemorySpace.PSUM) as wups,
            tc.tile_pool(name="ps32", bufs=2, space=bass.MemorySpace.PSUM) as ps32,
        ):
            # ---- constants / warmup (t=0, no input deps) ----
            wu_l = cst.tile([128, 128], BF16, tag="wul")
            nc.vector.memset(wu_l[:], 0.5)
            wu_r = cst.tile([128, 512], BF16, tag="wur")
            nc.vector.memset(wu_r[:], 0.5)
            cq128 = cst.tile([128, 128], F32, tag="cq128")
            nc.vector.memset(cq128[:], 1.0 / Cp)
            cx32 = cst.tile([C2, C2], F32, tag="cx32")
            nc.vector.memset(cx32[:], 1.0 / C2)
            eps32 = cst.tile([C2, 1], F32, tag="eps32")
            nc.vector.memset(eps32[:], EPS)
            eps128 = cst.tile([128, 1], F32, tag="eps128")
            nc.vector.memset(eps128[:], EPS)
            # preload the Sqrt activation table early (ACT, off critical path)
            atl = cst.tile([C2, 1], F32, tag="atl")
            nc.scalar.activation(atl[:], eps32[:], SQRT, bias=eps32[:])

            wu_ps = wups.tile([128, 512], F32, tag="wups")
            for i in range(NWU):
                nc.tensor.matmul(wu_ps[:], wu_l[:], wu_r[:], start=True, stop=True)

            def fillers(n):
                for _ in range(n):
                    nc.tensor.matmul(wu_ps[:], wu_l[:], wu_r[:], start=True, stop=True)

            # ---- input loads: x first (x-side chain is latency-critical) ----
            x_sb = cst.tile([C2, N], F32, tag="xs")
            charge("sync", dma_cost(4096))
            nc.sync.dma_start(x_sb[:], x_d[:])
            q_sb = []
            for k in range(8):
                qt = big.tile([128, N], BF16, tag=f"q{k}")
                q_sb.append(qt)
                eng = "sync" if k < 4 else "scalar"
                charge(eng, dma_cost(2048))
                getattr(nc, eng).dma_start(qt[:], q_d[128 * k : 128 * (k + 1), :])
            # w merged: w2[j][p, 256c+d] = wT[128(4j+c)+p, d]
            w_sb = []
            for j in range(2):
                wt = big.tile([128, 4 * Cp], BF16, tag=f"w{j}")
                w_sb.append(wt)
                src = w_d[512 * j : 512 * (j + 1), :].rearrange("(c p) d -> p c d", c=4)
                dst = wt[:].rearrange("p (c d) -> p c d", c=4)
                charge("gpsimd", dma_cost(2048))
                nc.gpsimd.dma_start(dst, src)

            def wslice(k, md):
                j, c = divmod(k, 4)
                return w_sb[j][:, 256 * c + 128 * md : 256 * c + 128 * (md + 1)]

            def cload(dram, shape, tag):
                t = cst.tile(shape, F32, tag=tag)
                eng = pick(DMA_ENGS, dma_cost(shape[1] * 4))
                getattr(nc, eng).dma_start(t[:], dram[:])
                return t

            if not simple:
                bp_sb = cload(bp_d, [128, 2], "bp")
                g1_sb = cload(g1_d, [128, 2], "g1")
                b1_sb = cload(b1_d, [128, 2], "b1")
                g2_sb = cload(g2_d, [C2, 1], "g2")
                b2_sb = cload(b2_d, [C2, 1], "b2")

            # ---- x-side LN (independent of q; A = xn ready early) ----
            xsq = cst.tile([C2, N], F32, tag="xsq")
            nc.gpsimd.tensor_mul(xsq[:], x_sb[:], x_sb[:])
            charge("gpsimd", N * 0.8333)

            mx = cst.tile([C2, N], F32, tag="mx")
            vx = cst.tile([C2, N], F32, tag="vx")
            sqxs = []
            for ci, (c0, c1) in enumerate(CHUNKS):
                w_ = c1 - c0
                smx = ps32.tile([C2, 512], F32, tag="s32")
                nc.tensor.matmul(
                    smx[:, :w_], cx32[:].bitcast(F32R), x_sb[:, c0:c1].bitcast(F32R),
                    start=True, stop=True,
                )
                sqx = ps32.tile([C2, 512], F32, tag="s32")
                nc.tensor.matmul(
                    sqx[:, :w_], cx32[:].bitcast(F32R), xsq[:, c0:c1].bitcast(F32R),
                    start=True, stop=True,
                )
                sqxs.append(sqx)
                nc.scalar.copy(mx[:, c0:c1], smx[:, :w_])
                charge("scalar", w_ * 0.8333 + 370)
            mx2 = cst.tile([C2, N], F32, tag="mx2")
            nc.gpsimd.tensor_mul(mx2[:], mx[:], mx[:])
            charge("gpsimd", N * 0.8333)
            for ci, (c0, c1) in enumerate(CHUNKS):
                w_ = c1 - c0
                nc.vector.tensor_sub(vx[:, c0:c1], sqxs[ci][:, :w_], mx2[:, c0:c1])
                charge("vector", w_ * 1.0417 + 125)
            sdx = cst.tile([C2, N], F32, tag="sdx")
            nc.scalar.activation(sdx[:], vx[:], SQRT, bias=eps32[:])
            charge("scalar", N * 0.8333 + 370)
            rsx = cst.tile([C2, N], F32, tag="rsx")
            nc.vector.reciprocal(rsx[:], sdx[:])
            charge("vector", N * 1.0417 + 60)
            xt = cst.tile([C2, N], F32, tag="xt")
            nc.gpsimd.tensor_sub(xt[:], x_sb[:], mx[:])
            charge("gpsimd", N * 0.8333)
            a_sb = cst.tile([C2, N], BF16, tag="a")
            if simple:
                nc.gpsimd.tensor_mul(a_sb[:], xt[:], rsx[:])
                charge("gpsimd", N * 0.8333)
            else:
                t3 = cst.tile([C2, N], F32, tag="t3")
                nc.gpsimd.tensor_mul(t3[:], xt[:], rsx[:])
                charge("gpsimd", N * 0.8333)
                nc.vector.tensor_scalar(
                    a_sb[:], t3[:], g2_sb[:, 0:1], b2_sb[:, 0:1], op0=MULT, op1=ADD
                )
                charge("vector", N * 1.0417 + 60)

            # A -> DRAM scratch, then all 32 broadcast reads (fills the DMA
            # window while the q-side LN chain runs)
            eng = pick(DMA_ENGS, dma_cost(N * 2))
            getattr(nc, eng).dma_start(abuf_d[:], a_sb[:])
            xbes = []
            for e in range(C2):
                t = xbp.tile([128, N], BF16, tag=f"xbe{e}")
                src = abuf_d[e : e + 1, :].partition_broadcast(128)
                eng = DMA_ENGS[e % 3]
                charge(eng, dma_cost(N * 2))
                getattr(nc, eng).dma_start(t[:], src)
                xbes.append(t)

            # ---- q-side: proj + LN -> cn (bf16, rstd folded in) ----
            cn = []
            for md in range(2):
                cnt = cst.tile([128, N], BF16, tag=f"cn{md}")
                cn.append(cnt)
            mb = cst.tile([128, N], F32, tag="mb")

            def qside_chunk(ci):
                c0, c1 = CHUNKS[ci]
                w_ = c1 - c0
                pj = []
                for md in range(2):
                    p_ = ps.tile([128, 512], F32, tag="ps")
                    for k in range(8):
                        nc.tensor.matmul(
                            p_[:, :w_], wslice(k, md), q_sb[k][:, c0:c1],
                            start=(k == 0), stop=(k == 7),
                        )
                    pj.append(p_)
                fillers(NFILL)

                projb, sq = [], []
                for md in range(2):
                    pb = wrk.tile([128, 512], F32, tag=f"pb{md}")
                    if simple:
                        nc.scalar.copy(pb[:, :w_], pj[md][:, :w_])
                    else:
                        nc.scalar.activation(
                            pb[:, :w_], pj[md][:, :w_], COPY,
                            bias=bp_sb[:, md : md + 1],
                        )
                    charge("scalar", w_ * 0.8333 + 370)
                    projb.append(pb)
                    s = wrk.tile([128, 512], F32, tag=f"sq{md}")
                    nc.gpsimd.tensor_mul(s[:, :w_], pb[:, :w_], pb[:, :w_])
                    charge("gpsimd", w_ * 0.8333)
                    sq.append(s)

                smq = ps.tile([128, 512], F32, tag="ps")
                for md in range(2):
                    nc.tensor.matmul(
                        smq[:, :w_], cq128[:].bitcast(F32R),
                        projb[md][:, :w_].bitcast(F32R),
                        start=(md == 0), stop=(md == 1),
                    )
                sqq = ps.tile([128, 512], F32, tag="ps")
                for md in range(2):
                    nc.tensor.matmul(
                        sqq[:, :w_], cq128[:].bitcast(F32R),
                        sq[md][:, :w_].bitcast(F32R),
                        start=(md == 0), stop=(md == 1),
                    )
                fillers(NFILL)

                nc.scalar.copy(mb[:, c0:c1], smq[:, :w_])
                charge("scalar", w_ * 0.8333 + 370)
                mb2 = wrk.tile([128, 512], F32, tag="mb2")
                nc.gpsimd.tensor_mul(mb2[:, :w_], mb[:, c0:c1], mb[:, c0:c1])
                charge("gpsimd", w_ * 0.8333)
                varq = wrk.tile([128, 512], F32, tag="varq")
                nc.vector.tensor_sub(varq[:, :w_], sqq[:, :w_], mb2[:, :w_])
                charge("vector", w_ * 1.0417 + 125)
                sdq = wrk.tile([128, 512], F32, tag="sdq")
                nc.scalar.activation(sdq[:, :w_], varq[:, :w_], SQRT, bias=eps128[:])
                charge("scalar", w_ * 0.8333 + 370)
                rstd = wrk.tile([128, 512], F32, tag="rstd")
                nc.vector.reciprocal(rstd[:, :w_], sdq[:, :w_])
                charge("vector", w_ * 1.0417 + 60)

                for md in range(2):
                    cs = wrk.tile([128, 512], F32, tag=f"cs{md}")
                    nc.gpsimd.tensor_sub(cs[:, :w_], projb[md][:, :w_], mb[:, c0:c1])
                    charge("gpsimd", w_ * 0.8333)
                    if simple:
                        nc.gpsimd.tensor_mul(cn[md][:, c0:c1], cs[:, :w_], rstd[:, :w_])
                        charge("gpsimd", w_ * 0.8333)
                    else:
                        c2_ = wrk.tile([128, 512], F32, tag=f"c2_{md}")
                        nc.gpsimd.tensor_mul(c2_[:, :w_], cs[:, :w_], rstd[:, :w_])
                        charge("gpsimd", w_ * 0.8333)
                        nc.vector.tensor_scalar(
                            cn[md][:, c0:c1], c2_[:, :w_],
                            g1_sb[:, md : md + 1], b1_sb[:, md : md + 1],
                            op0=MULT, op1=ADD,
                        )
                        charge("vector", w_ * 1.0417 + 60)

            POOL_E = {5}  # e % 8 in POOL_E -> Pool owns both products
            out_rr = [0]

            def emit_tile(e, c0, c1):
                """products + staging + out DMA for tile column range [c0,c1)."""
                w_ = c1 - c0
                if w_ == N:
                    st = stg.tile([128, 2 * N], BF16, tag="st")
                else:
                    st = stc.tile([128, 2 * 512], BF16, tag="stc")
                sw = st.shape[1] // 2
                meng = "gpsimd" if (e % 8) in POOL_E else "vector"
                for md in range(2):
                    charge(meng, mul_costs(w_)[meng])
                    getattr(nc, meng).tensor_mul(
                        st[:, sw * md : sw * md + w_],
                        cn[md][:, c0:c1],
                        xbes[e][:, c0:c1],
                    )
                dst = outv[:, :, e, c0:c1]
                src = st[:].rearrange("p (md n) -> p md n", md=2)[:, :, :w_]
                eng = ["sync", "scalar", "gpsimd", "sync", "scalar"][out_rr[0] % 5]
                out_rr[0] += 1
                charge(eng, dma_cost(2 * w_ * 2))
                getattr(nc, eng).dma_start(dst, src)

            qside_chunk(0)
            for e in range(ESPLIT):
                emit_tile(e, 0, 512)
            qside_chunk(1)
            for e in range(ESPLIT, C2):
                emit_tile(e, 0, N)
            for e in range(ESPLIT):
                emit_tile(e, 512, N)

    nc.compile()
    return nc


def _host_inputs(q, x, Wp, bp, g1, b1, g2, b2):
    """Build the 8 per-core input maps."""
    import ml_dtypes

    qf = np.asarray(q, dtype=np.float32).reshape(B, C1, N)
    qfb = qf.astype(ml_dtypes.bfloat16)
    xf = np.ascontiguousarray(np.asarray(x, dtype=np.float32).reshape(B, C2, N))
    wT = np.ascontiguousarray(np.asarray(Wp, dtype=np.float32).T).astype(
        ml_dtypes.bfloat16
    )
    bpc = np.ascontiguousarray(np.asarray(bp, dtype=np.float32).reshape(2, 128).T)
    g1c = np.ascontiguousarray(np.asarray(g1, dtype=np.float32).reshape(2, 128).T)
    b1c = np.ascontiguousarray(np.asarray(b1, dtype=np.float32).reshape(2, 128).T)
    g2r = np.ascontiguousarray(np.asarray(g2, dtype=np.float32)[:, None])
    b2r = np.ascontiguousarray(np.asarray(b2, dtype=np.float32)[:, None])
    in_maps = []
    for b in range(B):
        in_maps.append(
            {
                "qb": np.ascontiguousarray(qfb[b]),
                "wT": wT,
                "x": xf[b],
                "bpc": bpc,
                "g1c": g1c,
                "b1c": b1c,
                "g2r": g2r,
                "b2r": b2r,
            }
        )
    return in_maps


def _is_simple(bp, g1, b1, g2, b2):
    return (
        np.allclose(np.asarray(bp), 0)
        and np.allclose(np.asarray(g1), 1)
        and np.allclose(np.asarray(b1), 0)
        and np.allclose(np.asarray(g2), 1)
        and np.allclose(np.asarray(b2), 0)
    )


def _run(in_maps, simple=True, trace=False):
    from concourse.bass_utils import run_bass_kernel_spmd

    key = f"nc{int(simple)}"
    if key not in _CACHE:
        _CACHE[key] = _build_nc(simple)
    nc = _CACHE[key]
    res = run_bass_kernel_spmd(nc, in_maps, core_ids=list(range(B)), trace=trace)
    return res


def kernel(q, x, Wp, bp, g1, b1, g2, b2):
    simple = _is_simple(bp, g1, b1, g2, b2)
    _CACHE["simple"] = simple
    in_maps = _host_inputs(q, x, Wp, bp, g1, b1, g2, b2)
    res = _run(in_maps, simple=simple, trace=False)
    out = np.stack(
        [
            np.asarray(res.results[b]["out"]).astype(np.float32).reshape(CD, H, W)
            for b in range(B)
        ]
    )
    _CACHE["last_res"] = res
    return out


# revision 22
# speedup vs baseline: 1.1570x; 1.1570x over previous
"""Trainium2 Bass kernel for nn_HadaMard: fused proj + 2xLayerNorm + outer product.

Reference computation (per batch b, one NeuronCore per batch):
  qf = q[b].reshape(C1, N)           # [1024, 1024]
  proj = Wp @ qf (+ bp)              # [256, 1024]
  qn = LN_over_d(proj) * g1 + b1     # LN over the 256-channel dim
  xn = LN_over_e(x[b]) * g2 + b2     # LN over the 32-channel dim
  out[d*32+e, n] = qn[d, n] * xn[e, n]   # [8192, 1024]

Layout/strategy:
  - Output is transferred in bf16 (rel-err ~6e-3 << 2e-2 gate) and upcast on
    host: halves the dominant HBM write traffic.
  - Outer-product tiles are e-major: tile (md, e) holds out rows
    (128*md+p)*32+e for p in [0,128). The qn factor is the bf16 qn tile
    itself (no broadcast); the xn factor is one row broadcast to all 128
    partitions.
  - Row broadcasts go through a DRAM scratch roundtrip: A = xn is written
    once (ready early, x-side only), then each xbe tile is a stride-0
    partition-broadcast DMA read. These land on the DMA queues
    (sync/scalar/gpsimd) during the otherwise-idle window while the q-side
    LN chain runs, instead of loading the busy compute engines.
  - Elementwise products run on DVE (bf16 2x mode) and Pool, DMAs on
    sync/scalar/gpsimd, assigned by a static least-loaded balancer.
  - The q side is processed in two 512-column chunks (PSUM bank granularity);
    early e's run per-chunk products to start output DMA sooner, later e's
    run full-width products.
"""

import numpy as np

_CACHE = {}

B, C1, H, W = 8, 1024, 32, 32
C2 = 32
Cp = 256
N = H * W  # 1024
CD = Cp * C2  # 8192
EPS = 1e-5

ESPLIT = 8  # e < ESPLIT: per-chunk products; else full-width
NWU = 6  # PE warm-up matmuls
NFILL = 4  # PE filler matmuls between proj chunks


def _build_nc(simple):
    import concourse.bacc as bacc
    import concourse.bass as bass
    import concourse.mybir as mybir
    import concourse.tile as tile

    F32 = mybir.dt.float32
    F32R = mybir.dt.float32r
    BF16 = mybir.dt.bfloat16
    SQRT = mybir.ActivationFunctionType.Sqrt
    COPY = mybir.ActivationFunctionType.Copy
    MULT = mybir.AluOpType.mult
    ADD = mybir.AluOpType.add

    nc = bacc.Bacc(None, target_bir_lowering=False)

    q_d = nc.dram_tensor("qb", [C1, N], BF16, kind="ExternalInput")
    w_d = nc.dram_tensor("wT", [C1, Cp], BF16, kind="ExternalInput")
    x_d = nc.dram_tensor("x", [C2, N], F32, kind="ExternalInput")
    bp_d = nc.dram_tensor("bpc", [128, 2], F32, kind="ExternalInput")
    g1_d = nc.dram_tensor("g1c", [128, 2], F32, kind="ExternalInput")
    b1_d = nc.dram_tensor("b1c", [128, 2], F32, kind="ExternalInput")
    g2_d = nc.dram_tensor("g2r", [128, 1], F32, kind="ExternalInput")
    b2_d = nc.dram_tensor("b2r", [128, 1], F32, kind="ExternalInput")
    abuf_d = nc.dram_tensor("abuf", [128, 256], BF16, kind="Internal")  # packed: row 32a+e = A[e, 256a:]
    out_d = nc.dram_tensor("out", [CD, N], BF16, kind="ExternalOutput")

    # out view: row (md*128+p)*32+e  ->  [p, md, e, n]
    outv = out_d.rearrange("(md p e) n -> p md e n", md=2, p=128, e=C2)

    # ---- static least-loaded balancer (model-cost ns) ----
    clk = {"sync": 0.0, "scalar": 0.0, "gpsimd": 0.0, "vector": 0.0}

    def pick(cands, costs):
        e = min(cands, key=lambda x: clk[x])
        clk[e] += costs[e] if isinstance(costs, dict) else costs
        return e

    def charge(e, cost):
        clk[e] += cost

    DMA_ENGS = ["sync", "scalar", "gpsimd"]

    def dma_cost(bytes_per_part):
        return max(bytes_per_part * 0.3855, 500.0)

    def mul_costs(w):
        return {"vector": w * 1.0417 * 0.5 + 60.0, "gpsimd": w * 0.8333}

    CHUNKS = [(0, 512), (512, 1024)]

    with tile.TileContext(nc) as tc:
        with (
            tc.tile_pool(name="cst", bufs=1) as cst,
            tc.tile_pool(name="big", bufs=1) as big,
            tc.tile_pool(name="xbe", bufs=1) as xbp,
            tc.tile_pool(name="stg", bufs=7) as stg,
            tc.tile_pool(name="stc", bufs=4) as stc,
            tc.tile_pool(name="wrk", bufs=2) as wrk,
            tc.tile_pool(name="ps", bufs=4, space=bass.MemorySpace.PSUM) as ps,
            tc.tile_pool(name="wups", bufs=1, space=bass.MemorySpace.PSUM) as wups,
            tc.tile_pool(name="ps32", bufs=2, space=bass.MemorySpace.PSUM) as ps32,
        ):
            # ---- constants / warmup (t=0, no input deps) ----
            wu_l = cst.tile([128, 128], BF16, tag="wul")
            nc.vector.memset(wu_l[:], 0.5)
            wu_r = cst.tile([128, 512], BF16, tag="wur")
            nc.vector.memset(wu_r[:], 0.5)
            cq128 = cst.tile([128, 128], F32, tag="cq128")
            nc.vector.memset(cq128[:], 1.0 / Cp)
            cjx = cst.tile([128, 128], F32, tag="cjx")
            nc.vector.memset(cjx[:], 0.0)
            for a_ in range(4):
                nc.vector.memset(cjx[32 * a_ : 32 * (a_ + 1), 32 * a_ : 32 * (a_ + 1)], 1.0 / C2)
            eps32 = cst.tile([C2, 1], F32, tag="eps32")
            nc.vector.memset(eps32[:], EPS)
            eps128 = cst.tile([128, 1], F32, tag="eps128")
            nc.vector.memset(eps128[:], EPS)
            # preload the Sqrt activation table early (ACT, off critical path)
            atl = cst.tile([C2, 1], F32, tag="atl")
            nc.scalar.activation(atl[:], eps32[:], SQRT, bias=eps32[:])

            wu_ps = wups.tile([128, 512], F32, tag="wups")
            for i in range(NWU):
                nc.tensor.matmul(wu_ps[:], wu_l[:], wu_r[:], start=True, stop=True)

            def fillers(n):
                for _ in range(n):
                    nc.tensor.matmul(wu_ps[:], wu_l[:], wu_r[:], start=True, stop=True)

            # ---- input loads: x first (x-side chain is latency-critical) ----
            # packed layout: x4[32a+e, n'] = x[e, 256a+n']
            x_sb = cst.tile([128, 256], F32, tag="xs")
            charge("sync", dma_cost(1024))
            nc.sync.dma_start(
                x_sb[:], x_d.rearrange("e (a n) -> a e n", a=4)
            )
            q_sb = []
            for k in range(8):
                qt = big.tile([128, N], BF16, tag=f"q{k}")
                q_sb.append(qt)
                eng = "sync" if k < 4 else "scalar"
                charge(eng, dma_cost(2048))
                getattr(nc, eng).dma_start(qt[:], q_d[128 * k : 128 * (k + 1), :])
            # w merged: w2[j][p, 256c+d] = wT[128(4j+c)+p, d]
            w_sb = []
            for j in range(2):
                wt = big.tile([128, 4 * Cp], BF16, tag=f"w{j}")
                w_sb.append(wt)
                src = w_d[512 * j : 512 * (j + 1), :].rearrange("(c p) d -> p c d", c=4)
                dst = wt[:].rearrange("p (c d) -> p c d", c=4)
                charge("gpsimd", dma_cost(2048))
                nc.gpsimd.dma_start(dst, src)

            def wslice(k, md):
                j, c = divmod(k, 4)
                return w_sb[j][:, 256 * c + 128 * md : 256 * c + 128 * (md + 1)]

            def cload(dram, shape, tag):
                t = cst.tile(shape, F32, tag=tag)
                eng = pick(DMA_ENGS, dma_cost(shape[1] * 4))
                getattr(nc, eng).dma_start(t[:], dram[:])
                return t

            if not simple:
                bp_sb = cload(bp_d, [128, 2], "bp")
                g1_sb = cload(g1_d, [128, 2], "g1")
                b1_sb = cload(b1_d, [128, 2], "b1")
                g2_sb = cload(g2_d, [128, 1], "g2")
                b2_sb = cload(b2_d, [128, 1], "b2")

            # ---- x-side LN (packed [128,256]; independent of q) ----
            xsq = cst.tile([128, 256], F32, tag="xsq")
            nc.gpsimd.tensor_mul(xsq[:], x_sb[:], x_sb[:])
            charge("gpsimd", 256 * 0.8333)
            smx = ps32.tile([128, 256], F32, tag="s32")
            nc.tensor.matmul(
                smx[:], cjx[:].bitcast(F32R), x_sb[:].bitcast(F32R),
                start=True, stop=True,
            )
            sqx = ps32.tile([128, 256], F32, tag="s32")
            nc.tensor.matmul(
                sqx[:], cjx[:].bitcast(F32R), xsq[:].bitcast(F32R),
                start=True, stop=True,
            )
            mx = cst.tile([128, 256], F32, tag="mx")
            nc.scalar.copy(mx[:], smx[:])
            charge("scalar", 256 * 0.8333 + 370)
            mx2 = cst.tile([128, 256], F32, tag="mx2")
            nc.gpsimd.tensor_mul(mx2[:], mx[:], mx[:])
            charge("gpsimd", 256 * 0.8333)
            vx = cst.tile([128, 256], F32, tag="vx")
            nc.vector.tensor_sub(vx[:], sqx[:], mx2[:])
            charge("vector", 256 * 1.0417 + 125)
            sdx = cst.tile([128, 256], F32, tag="sdx")
            nc.scalar.activation(sdx[:], vx[:], SQRT, bias=eps128[:])
            charge("scalar", 256 * 0.8333 + 370)
            rsx = cst.tile([128, 256], F32, tag="rsx")
            nc.vector.reciprocal(rsx[:], sdx[:])
            charge("vector", 256 * 1.0417 + 60)
            xt = cst.tile([128, 256], F32, tag="xt")
            nc.gpsimd.tensor_sub(xt[:], x_sb[:], mx[:])
            charge("gpsimd", 256 * 0.8333)
            a_sb = cst.tile([128, 256], BF16, tag="a")
            if simple:
                nc.gpsimd.tensor_mul(a_sb[:], xt[:], rsx[:])
                charge("gpsimd", 256 * 0.8333)
            else:
                t3 = cst.tile([128, 256], F32, tag="t3")
                nc.gpsimd.tensor_mul(t3[:], xt[:], rsx[:])
                charge("gpsimd", 256 * 0.8333)
                nc.vector.tensor_scalar(
                    a_sb[:], t3[:], g2_sb[:, 0:1], b2_sb[:, 0:1], op0=MULT, op1=ADD
                )
                charge("vector", 256 * 1.0417 + 60)

            # A -> DRAM scratch (packed), then all 32 broadcast reads (fill
            # the DMA window while the q-side LN chain runs)
            charge("sync", dma_cost(512))
            nc.sync.dma_start(abuf_d[:], a_sb[:])
            abufv = abuf_d.rearrange("(a e) n -> e a n", a=4, e=C2)
            xbes = []
            for e in range(C2):
                t = xbp.tile([128, N], BF16, tag=f"xbe{e}")
                src = abufv[e : e + 1, :, :].partition_broadcast(128)
                eng = DMA_ENGS[e % 3]
                charge(eng, dma_cost(N * 2))
                getattr(nc, eng).dma_start(
                    t[:].rearrange("p (a n) -> p a n", a=4), src
                )
                xbes.append(t)

            # ---- q-side: proj + LN -> cn (bf16, rstd folded in) ----
            cn = []
            for md in range(2):
                cnt = cst.tile([128, N], BF16, tag=f"cn{md}")
                cn.append(cnt)
            mb = cst.tile([128, N], F32, tag="mb")

            def qside_chunk(ci):
                c0, c1 = CHUNKS[ci]
                w_ = c1 - c0
                pj = []
                for md in range(2):
                    p_ = ps.tile([128, 512], F32, tag="ps")
                    for k in range(8):
                        nc.tensor.matmul(
                            p_[:, :w_], wslice(k, md), q_sb[k][:, c0:c1],
                            start=(k == 0), stop=(k == 7),
                        )
                    pj.append(p_)
                fillers(NFILL)

                projb, sq = [], []
                for md in range(2):
                    pb = wrk.tile([128, 512], F32, tag=f"pb{md}")
                    if simple:
                        nc.scalar.copy(pb[:, :w_], pj[md][:, :w_])
                    else:
                        nc.scalar.activation(
                            pb[:, :w_], pj[md][:, :w_], COPY,
                            bias=bp_sb[:, md : md + 1],
                        )
                    charge("scalar", w_ * 0.8333 + 370)
                    projb.append(pb)
                    s = wrk.tile([128, 512], F32, tag=f"sq{md}")
                    nc.gpsimd.tensor_mul(s[:, :w_], pb[:, :w_], pb[:, :w_])
                    charge("gpsimd", w_ * 0.8333)
                    sq.append(s)

                smq = ps.tile([128, 512], F32, tag="ps")
                for md in range(2):
                    nc.tensor.matmul(
                        smq[:, :w_], cq128[:].bitcast(F32R),
                        projb[md][:, :w_].bitcast(F32R),
                        start=(md == 0), stop=(md == 1),
                    )
                sqq = ps.tile([128, 512], F32, tag="ps")
                for md in range(2):
                    nc.tensor.matmul(
                        sqq[:, :w_], cq128[:].bitcast(F32R),
                        sq[md][:, :w_].bitcast(F32R),
                        start=(md == 0), stop=(md == 1),
                    )
                fillers(NFILL)

                nc.scalar.copy(mb[:, c0:c1], smq[:, :w_])
                charge("scalar", w_ * 0.8333 + 370)
                mb2 = wrk.tile([128, 512], F32, tag="mb2")
                nc.gpsimd.tensor_mul(mb2[:, :w_], mb[:, c0:c1], mb[:, c0:c1])
                charge("gpsimd", w_ * 0.8333)
                varq = wrk.tile([128, 512], F32, tag="varq")
                nc.vector.tensor_sub(varq[:, :w_], sqq[:, :w_], mb2[:, :w_])
                charge("vector", w_ * 1.0417 + 125)
                sdq = wrk.tile([128, 512], F32, tag="sdq")
                nc.scalar.activation(sdq[:, :w_], varq[:, :w_], SQRT, bias=eps128[:])
                charge("scalar", w_ * 0.8333 + 370)
                rstd = wrk.tile([128, 512], F32, tag="rstd")
                nc.vector.reciprocal(rstd[:, :w_], sdq[:, :w_])
                charge("vector", w_ * 1.0417 + 60)

                for md in range(2):
                    cs = wrk.tile([128, 512], F32, tag=f"cs{md}")
                    nc.gpsimd.tensor_sub(cs[:, :w_], projb[md][:, :w_], mb[:, c0:c1])
                    charge("gpsimd", w_ * 0.8333)
                    if simple:
                        nc.gpsimd.tensor_mul(cn[md][:, c0:c1], cs[:, :w_], rstd[:, :w_])
                        charge("gpsimd", w_ * 0.8333)
                    else:
                        c2_ = wrk.tile([128, 512], F32, tag=f"c2_{md}")
                        nc.gpsimd.tensor_mul(c2_[:, :w_], cs[:, :w_], rstd[:, :w_])
                        charge("gpsimd", w_ * 0.8333)
                        nc.vector.tensor_scalar(
                            cn[md][:, c0:c1], c2_[:, :w_],
                            g1_sb[:, md : md + 1], b1_sb[:, md : md + 1],
                            op0=MULT, op1=ADD,
                        )
                        charge("vector", w_ * 1.0417 + 60)

            POOL_E = {2, 5, 7}  # e % 8 in POOL_E -> Pool owns both products
            out_rr = [0]

            def emit_tile(e, c0, c1):
                """products + staging + out DMA for tile column range [c0,c1)."""
                w_ = c1 - c0
                if w_ == N:
                    st = stg.tile([128, 2 * N], BF16, tag="st")
                else:
                    st = stc.tile([128, 2 * 512], BF16, tag="stc")
                sw = st.shape[1] // 2
                meng = "gpsimd" if (e % 8) in POOL_E else "vector"
                for md in range(2):
                    charge(meng, mul_costs(w_)[meng])
                    getattr(nc, meng).tensor_mul(
                        st[:, sw * md : sw * md + w_],
                        cn[md][:, c0:c1],
                        xbes[e][:, c0:c1],
                    )
                dst = outv[:, :, e, c0:c1]
                src = st[:].rearrange("p (md n) -> p md n", md=2)[:, :, :w_]
                eng = ["sync", "scalar"][out_rr[0] % 2]
                out_rr[0] += 1
                charge(eng, dma_cost(2 * w_ * 2))
                getattr(nc, eng).dma_start(dst, src)

            qside_chunk(0)
            for e in range(ESPLIT):
                emit_tile(e, 0, 512)
            qside_chunk(1)
            for e in range(ESPLIT, C2):
                emit_tile(e, 0, N)
            for e in range(ESPLIT):
                emit_tile(e, 512, N)

    nc.compile()
    return nc


def _host_inputs(q, x, Wp, bp, g1, b1, g2, b2):
    """Build the 8 per-core input maps."""
    import ml_dtypes

    qf = np.asarray(q, dtype=np.float32).reshape(B, C1, N)
    qfb = qf.astype(ml_dtypes.bfloat16)
    xf = np.ascontiguousarray(np.asarray(x, dtype=np.float32).reshape(B, C2, N))
    wT = np.ascontiguousarray(np.asarray(Wp, dtype=np.float32).T).astype(
        ml_dtypes.bfloat16
    )
    bpc = np.ascontiguousarray(np.asarray(bp, dtype=np.float32).reshape(2, 128).T)
    g1c = np.ascontiguousarray(np.asarray(g1, dtype=np.float32).reshape(2, 128).T)
    b1c = np.ascontiguousarray(np.asarray(b1, dtype=np.float32).reshape(2, 128).T)
    g2r = np.ascontiguousarray(np.tile(np.asarray(g2, dtype=np.float32), 4)[:, None])
    b2r = np.ascontiguousarray(np.tile(np.asarray(b2, dtype=np.float32), 4)[:, None])
    in_maps = []
    for b in range(B):
        in_maps.append(
            {
                "qb": np.ascontiguousarray(qfb[b]),
                "wT": wT,
                "x": xf[b],
                "bpc": bpc,
                "g1c": g1c,
                "b1c": b1c,
                "g2r": g2r,
                "b2r": b2r,
            }
        )
    return in_maps


def _is_simple(bp, g1, b1, g2, b2):
    return (
        np.allclose(np.asarray(bp), 0)
        and np.allclose(np.asarray(g1), 1)
        and np.allclose(np.asarray(b1), 0)
        and np.allclose(np.asarray(g2), 1)
        and np.allclose(np.asarray(b2), 0)
    )


def _run(in_maps, simple=True, trace=False):
    from concourse.bass_utils import run_bass_kernel_spmd

    key = f"nc{int(simple)}"
    if key not in _CACHE:
        _CACHE[key] = _build_nc(simple)
    nc = _CACHE[key]
    res = run_bass_kernel_spmd(nc, in_maps, core_ids=list(range(B)), trace=trace)
    return res


def kernel(q, x, Wp, bp, g1, b1, g2, b2):
    simple = _is_simple(bp, g1, b1, g2, b2)
    _CACHE["simple"] = simple
    in_maps = _host_inputs(q, x, Wp, bp, g1, b1, g2, b2)
    res = _run(in_maps, simple=simple, trace=False)
    out = np.stack(
        [
            np.asarray(res.results[b]["out"]).astype(np.float32).reshape(CD, H, W)
            for b in range(B)
        ]
    )
    _CACHE["last_res"] = res
    return out


# revision 23
# speedup vs baseline: 1.1807x; 1.0205x over previous
"""Trainium2 Bass kernel for nn_HadaMard: fused proj + 2xLayerNorm + outer product.

Reference computation (per batch b, one NeuronCore per batch):
  qf = q[b].reshape(C1, N)           # [1024, 1024]
  proj = Wp @ qf (+ bp)              # [256, 1024]
  qn = LN_over_d(proj) * g1 + b1     # LN over the 256-channel dim
  xn = LN_over_e(x[b]) * g2 + b2     # LN over the 32-channel dim
  out[d*32+e, n] = qn[d, n] * xn[e, n]   # [8192, 1024]

Layout/strategy:
  - Output is transferred in bf16 (rel-err ~6e-3 << 2e-2 gate) and upcast on
    host: halves the dominant HBM write traffic.
  - Outer-product tiles are e-major: tile (md, e) holds out rows
    (128*md+p)*32+e for p in [0,128). The qn factor is the bf16 qn tile
    itself (no broadcast); the xn factor is one row broadcast to all 128
    partitions.
  - Row broadcasts go through a DRAM scratch roundtrip: A = xn is written
    once (ready early, x-side only), then each xbe tile is a stride-0
    partition-broadcast DMA read. These land on the DMA queues
    (sync/scalar/gpsimd) during the otherwise-idle window while the q-side
    LN chain runs, instead of loading the busy compute engines.
  - Elementwise products run on DVE (bf16 2x mode) and Pool, DMAs on
    sync/scalar/gpsimd, assigned by a static least-loaded balancer.
  - The q side is processed in two 512-column chunks (PSUM bank granularity);
    early e's run per-chunk products to start output DMA sooner, later e's
    run full-width products.
"""

import numpy as np

_CACHE = {}

B, C1, H, W = 8, 1024, 32, 32
C2 = 32
Cp = 256
N = H * W  # 1024
CD = Cp * C2  # 8192
EPS = 1e-5

ESPLIT = 4  # e < ESPLIT: per-chunk products; else full-width
NWU = 3  # PE warm-up matmuls
NFILL = 4  # PE filler matmuls between proj chunks


def _build_nc(simple):
    import concourse.bacc as bacc
    import concourse.bass as bass
    import concourse.mybir as mybir
    import concourse.tile as tile

    F32 = mybir.dt.float32
    F32R = mybir.dt.float32r
    BF16 = mybir.dt.bfloat16
    SQRT = mybir.ActivationFunctionType.Sqrt
    COPY = mybir.ActivationFunctionType.Copy
    MULT = mybir.AluOpType.mult
    ADD = mybir.AluOpType.add

    nc = bacc.Bacc(None, target_bir_lowering=False)

    q_d = nc.dram_tensor("qb", [C1, N], BF16, kind="ExternalInput")
    w_d = nc.dram_tensor("wT", [C1, Cp], BF16, kind="ExternalInput")
    x_d = nc.dram_tensor("x", [C2, N], F32, kind="ExternalInput")
    bp_d = nc.dram_tensor("bpc", [128, 2], F32, kind="ExternalInput")
    g1_d = nc.dram_tensor("g1c", [128, 2], F32, kind="ExternalInput")
    b1_d = nc.dram_tensor("b1c", [128, 2], F32, kind="ExternalInput")
    g2_d = nc.dram_tensor("g2r", [128, 1], F32, kind="ExternalInput")
    b2_d = nc.dram_tensor("b2r", [128, 1], F32, kind="ExternalInput")
    abuf_d = nc.dram_tensor("abuf", [128, 256], BF16, kind="Internal")  # packed: row 32a+e = A[e, 256a:]
    out_d = nc.dram_tensor("out", [CD, N], BF16, kind="ExternalOutput")

    # out view: row (md*128+p)*32+e  ->  [p, md, e, n]
    outv = out_d.rearrange("(md p e) n -> p md e n", md=2, p=128, e=C2)

    # ---- static least-loaded balancer (model-cost ns) ----
    clk = {"sync": 0.0, "scalar": 0.0, "gpsimd": 0.0, "vector": 0.0}

    def pick(cands, costs):
        e = min(cands, key=lambda x: clk[x])
        clk[e] += costs[e] if isinstance(costs, dict) else costs
        return e

    def charge(e, cost):
        clk[e] += cost

    DMA_ENGS = ["sync", "scalar", "gpsimd"]

    def dma_cost(bytes_per_part):
        return max(bytes_per_part * 0.3855, 500.0)

    def mul_costs(w):
        return {"vector": w * 1.0417 * 0.5 + 60.0, "gpsimd": w * 0.8333}

    CHUNKS = [(0, 512), (512, 1024)]

    with tile.TileContext(nc) as tc:
        with (
            tc.tile_pool(name="cst", bufs=1) as cst,
            tc.tile_pool(name="big", bufs=1) as big,
            tc.tile_pool(name="xbe", bufs=1) as xbp,
            tc.tile_pool(name="stg", bufs=7) as stg,
            tc.tile_pool(name="stc", bufs=4) as stc,
            tc.tile_pool(name="wrk", bufs=2) as wrk,
            tc.tile_pool(name="ps", bufs=4, space=bass.MemorySpace.PSUM) as ps,
            tc.tile_pool(name="wups", bufs=1, space=bass.MemorySpace.PSUM) as wups,
            tc.tile_pool(name="ps32", bufs=2, space=bass.MemorySpace.PSUM) as ps32,
        ):
            # ---- constants / warmup (t=0, no input deps) ----
            wu_l = cst.tile([128, 128], BF16, tag="wul")
            nc.vector.memset(wu_l[:], 0.5)
            wu_r = cst.tile([128, 512], BF16, tag="wur")
            nc.vector.memset(wu_r[:], 0.5)
            cq128 = cst.tile([128, 128], F32, tag="cq128")
            nc.vector.memset(cq128[:], 1.0 / Cp)
            cjx = cst.tile([128, 128], F32, tag="cjx")
            nc.vector.memset(cjx[:], 0.0)
            for a_ in range(4):
                nc.vector.memset(cjx[32 * a_ : 32 * (a_ + 1), 32 * a_ : 32 * (a_ + 1)], 1.0 / C2)
            eps32 = cst.tile([C2, 1], F32, tag="eps32")
            nc.vector.memset(eps32[:], EPS)
            eps128 = cst.tile([128, 1], F32, tag="eps128")
            nc.vector.memset(eps128[:], EPS)
            # preload the activation table early (ACT, off critical path)
            atl = cst.tile([C2, 1], F32, tag="atl")
            nc.scalar.copy(atl[:], eps32[:])
            nc.scalar.activation(atl[:], eps32[:], SQRT, bias=eps32[:])

            wu_ps = wups.tile([128, 512], F32, tag="wups")
            for i in range(NWU):
                nc.tensor.matmul(wu_ps[:], wu_l[:], wu_r[:], start=True, stop=True)

            def fillers(n):
                for _ in range(n):
                    nc.tensor.matmul(wu_ps[:], wu_l[:], wu_r[:], start=True, stop=True)

            # ---- input loads: x first (x-side chain is latency-critical) ----
            # packed layout: x4[32a+e, n'] = x[e, 256a+n']
            x_sb = cst.tile([128, 256], F32, tag="xs")
            charge("sync", dma_cost(1024))
            nc.sync.dma_start(
                x_sb[:], x_d.rearrange("e (a n) -> a e n", a=4)
            )
            q_sb = []
            for k in range(8):
                qt = big.tile([128, N], BF16, tag=f"q{k}")
                q_sb.append(qt)
                eng = "sync" if k < 4 else "scalar"
                charge(eng, dma_cost(2048))
                getattr(nc, eng).dma_start(qt[:], q_d[128 * k : 128 * (k + 1), :])
            # w merged: w2[j][p, 256c+d] = wT[128(4j+c)+p, d]
            w_sb = []
            for j in range(2):
                wt = big.tile([128, 4 * Cp], BF16, tag=f"w{j}")
                w_sb.append(wt)
                src = w_d[512 * j : 512 * (j + 1), :].rearrange("(c p) d -> p c d", c=4)
                dst = wt[:].rearrange("p (c d) -> p c d", c=4)
                charge("gpsimd", dma_cost(2048))
                nc.gpsimd.dma_start(dst, src)

            def wslice(k, md):
                j, c = divmod(k, 4)
                return w_sb[j][:, 256 * c + 128 * md : 256 * c + 128 * (md + 1)]

            def cload(dram, shape, tag):
                t = cst.tile(shape, F32, tag=tag)
                eng = pick(DMA_ENGS, dma_cost(shape[1] * 4))
                getattr(nc, eng).dma_start(t[:], dram[:])
                return t

            if not simple:
                bp_sb = cload(bp_d, [128, 2], "bp")
                g1_sb = cload(g1_d, [128, 2], "g1")
                b1_sb = cload(b1_d, [128, 2], "b1")
                g2_sb = cload(g2_d, [128, 1], "g2")
                b2_sb = cload(b2_d, [128, 1], "b2")

            # ---- x-side LN (packed [128,256]; independent of q) ----
            xsq = cst.tile([128, 256], F32, tag="xsq")
            nc.gpsimd.tensor_mul(xsq[:], x_sb[:], x_sb[:])
            charge("gpsimd", 256 * 0.8333)
            smx = ps32.tile([128, 256], F32, tag="s32")
            nc.tensor.matmul(
                smx[:], cjx[:].bitcast(F32R), x_sb[:].bitcast(F32R),
                start=True, stop=True,
            )
            sqx = ps32.tile([128, 256], F32, tag="s32")
            nc.tensor.matmul(
                sqx[:], cjx[:].bitcast(F32R), xsq[:].bitcast(F32R),
                start=True, stop=True,
            )
            mx = cst.tile([128, 256], F32, tag="mx")
            nc.scalar.copy(mx[:], smx[:])
            charge("scalar", 256 * 0.8333 + 370)
            mx2 = cst.tile([128, 256], F32, tag="mx2")
            nc.gpsimd.tensor_mul(mx2[:], mx[:], mx[:])
            charge("gpsimd", 256 * 0.8333)
            vx = cst.tile([128, 256], F32, tag="vx")
            nc.vector.tensor_sub(vx[:], sqx[:], mx2[:])
            charge("vector", 256 * 1.0417 + 125)
            sdx = cst.tile([128, 256], F32, tag="sdx")
            nc.scalar.activation(sdx[:], vx[:], SQRT, bias=eps128[:])
            charge("scalar", 256 * 0.8333 + 370)
            rsx = cst.tile([128, 256], F32, tag="rsx")
            nc.vector.reciprocal(rsx[:], sdx[:])
            charge("vector", 256 * 1.0417 + 60)
            xt = cst.tile([128, 256], F32, tag="xt")
            nc.gpsimd.tensor_sub(xt[:], x_sb[:], mx[:])
            charge("gpsimd", 256 * 0.8333)
            a_sb = cst.tile([128, 256], BF16, tag="a")
            if simple:
                nc.gpsimd.tensor_mul(a_sb[:], xt[:], rsx[:])
                charge("gpsimd", 256 * 0.8333)
            else:
                t3 = cst.tile([128, 256], F32, tag="t3")
                nc.gpsimd.tensor_mul(t3[:], xt[:], rsx[:])
                charge("gpsimd", 256 * 0.8333)
                nc.vector.tensor_scalar(
                    a_sb[:], t3[:], g2_sb[:, 0:1], b2_sb[:, 0:1], op0=MULT, op1=ADD
                )
                charge("vector", 256 * 1.0417 + 60)

            # A -> DRAM scratch (packed), then all 32 broadcast reads (fill
            # the DMA window while the q-side LN chain runs)
            charge("sync", dma_cost(512))
            nc.sync.dma_start(abuf_d[:], a_sb[:])
            abufv = abuf_d.rearrange("(a e) n -> e a n", a=4, e=C2)
            xbes = [None] * C2
            xbe_rr = [0]

            def emit_xbe(e, eng=None):
                if xbes[e] is not None:
                    return
                t = xbp.tile([128, N], BF16, tag=f"xbe{e}")
                src = abufv[e : e + 1, :, :].partition_broadcast(128)
                if eng is None:
                    eng = ["scalar", "gpsimd", "sync"][xbe_rr[0] % 3]
                    xbe_rr[0] += 1
                charge(eng, dma_cost(N * 2))
                getattr(nc, eng).dma_start(
                    t[:].rearrange("p (a n) -> p a n", a=4), src
                )
                xbes[e] = t

            for e in range(6):
                emit_xbe(e, "sync")

            # ---- q-side: proj + LN -> cn (bf16, rstd folded in) ----
            cn = []
            for md in range(2):
                cnt = cst.tile([128, N], BF16, tag=f"cn{md}")
                cn.append(cnt)
            mb = cst.tile([128, N], F32, tag="mb")

            def qside_chunk(ci):
                c0, c1 = CHUNKS[ci]
                w_ = c1 - c0
                pj = []
                for md in range(2):
                    p_ = ps.tile([128, 512], F32, tag="ps")
                    for k in range(8):
                        nc.tensor.matmul(
                            p_[:, :w_], wslice(k, md), q_sb[k][:, c0:c1],
                            start=(k == 0), stop=(k == 7),
                        )
                    pj.append(p_)
                fillers(NFILL)

                projb, sq = [], []
                for md in range(2):
                    pb = wrk.tile([128, 512], F32, tag=f"pb{md}")
                    if simple:
                        nc.scalar.copy(pb[:, :w_], pj[md][:, :w_])
                    else:
                        nc.scalar.activation(
                            pb[:, :w_], pj[md][:, :w_], COPY,
                            bias=bp_sb[:, md : md + 1],
                        )
                    charge("scalar", w_ * 0.8333 + 370)
                    projb.append(pb)
                    s = wrk.tile([128, 512], F32, tag=f"sq{md}")
                    nc.gpsimd.tensor_mul(s[:, :w_], pb[:, :w_], pb[:, :w_])
                    charge("gpsimd", w_ * 0.8333)
                    sq.append(s)

                smq = ps.tile([128, 512], F32, tag="ps")
                for md in range(2):
                    nc.tensor.matmul(
                        smq[:, :w_], cq128[:].bitcast(F32R),
                        projb[md][:, :w_].bitcast(F32R),
                        start=(md == 0), stop=(md == 1),
                    )
                sqq = ps.tile([128, 512], F32, tag="ps")
                for md in range(2):
                    nc.tensor.matmul(
                        sqq[:, :w_], cq128[:].bitcast(F32R),
                        sq[md][:, :w_].bitcast(F32R),
                        start=(md == 0), stop=(md == 1),
                    )
                fillers(NFILL)

                nc.scalar.copy(mb[:, c0:c1], smq[:, :w_])
                charge("scalar", w_ * 0.8333 + 370)
                mb2 = wrk.tile([128, 512], F32, tag="mb2")
                nc.gpsimd.tensor_mul(mb2[:, :w_], mb[:, c0:c1], mb[:, c0:c1])
                charge("gpsimd", w_ * 0.8333)
                varq = wrk.tile([128, 512], F32, tag="varq")
                nc.vector.tensor_sub(varq[:, :w_], sqq[:, :w_], mb2[:, :w_])
                charge("vector", w_ * 1.0417 + 125)
                sdq = wrk.tile([128, 512], F32, tag="sdq")
                nc.scalar.activation(sdq[:, :w_], varq[:, :w_], SQRT, bias=eps128[:])
                charge("scalar", w_ * 0.8333 + 370)
                rstd = wrk.tile([128, 512], F32, tag="rstd")
                nc.vector.reciprocal(rstd[:, :w_], sdq[:, :w_])
                charge("vector", w_ * 1.0417 + 60)

                for md in range(2):
                    cs = wrk.tile([128, 512], F32, tag=f"cs{md}")
                    nc.gpsimd.tensor_sub(cs[:, :w_], projb[md][:, :w_], mb[:, c0:c1])
                    charge("gpsimd", w_ * 0.8333)
                    if simple:
                        nc.gpsimd.tensor_mul(cn[md][:, c0:c1], cs[:, :w_], rstd[:, :w_])
                        charge("gpsimd", w_ * 0.8333)
                    else:
                        c2_ = wrk.tile([128, 512], F32, tag=f"c2_{md}")
                        nc.gpsimd.tensor_mul(c2_[:, :w_], cs[:, :w_], rstd[:, :w_])
                        charge("gpsimd", w_ * 0.8333)
                        nc.vector.tensor_scalar(
                            cn[md][:, c0:c1], c2_[:, :w_],
                            g1_sb[:, md : md + 1], b1_sb[:, md : md + 1],
                            op0=MULT, op1=ADD,
                        )
                        charge("vector", w_ * 1.0417 + 60)

            POOL_E = {2, 5, 7}  # e % 8 in POOL_E -> Pool owns both products
            out_rr = [0]

            def emit_tile(e, c0, c1, force_eng=None, force_out=None):
                """products + staging + out DMA for tile column range [c0,c1)."""
                w_ = c1 - c0
                if w_ == N:
                    st = stg.tile([128, 2 * N], BF16, tag="st")
                else:
                    st = stc.tile([128, 2 * 512], BF16, tag="stc")
                sw = st.shape[1] // 2
                meng = force_eng or ("gpsimd" if (e % 8) in POOL_E else "vector")
                for md in range(2):
                    charge(meng, mul_costs(w_)[meng])
                    getattr(nc, meng).tensor_mul(
                        st[:, sw * md : sw * md + w_],
                        cn[md][:, c0:c1],
                        xbes[e][:, c0:c1],
                    )
                dst = outv[:, :, e, c0:c1]
                src = st[:].rearrange("p (md n) -> p md n", md=2)[:, :, :w_]
                if force_out is not None:
                    eng = force_out
                else:
                    eng = ["sync", "scalar", "gpsimd", "sync", "scalar"][out_rr[0] % 5]
                    out_rr[0] += 1
                charge(eng, dma_cost(2 * w_ * 2))
                getattr(nc, eng).dma_start(dst, src)

            qside_chunk(0)
            # head tiles: DVE products, sync outs (other queues hold LN ops)
            for e in range(ESPLIT):
                emit_tile(e, 0, 512, force_eng="vector", force_out="sync")
            qside_chunk(1)
            for e in range(ESPLIT, C2):
                emit_xbe(min(e + 2, C2 - 1))
                emit_tile(e, 0, N)
            for e in range(ESPLIT):
                emit_tile(e, 512, N)

    nc.compile()
    return nc


def _host_inputs(q, x, Wp, bp, g1, b1, g2, b2):
    """Build the 8 per-core input maps."""
    import ml_dtypes

    qf = np.asarray(q, dtype=np.float32).reshape(B, C1, N)
    qfb = qf.astype(ml_dtypes.bfloat16)
    xf = np.ascontiguousarray(np.asarray(x, dtype=np.float32).reshape(B, C2, N))
    wT = np.ascontiguousarray(np.asarray(Wp, dtype=np.float32).T).astype(
        ml_dtypes.bfloat16
    )
    bpc = np.ascontiguousarray(np.asarray(bp, dtype=np.float32).reshape(2, 128).T)
    g1c = np.ascontiguousarray(np.asarray(g1, dtype=np.float32).reshape(2, 128).T)
    b1c = np.ascontiguousarray(np.asarray(b1, dtype=np.float32).reshape(2, 128).T)
    g2r = np.ascontiguousarray(np.tile(np.asarray(g2, dtype=np.float32), 4)[:, None])
    b2r = np.ascontiguousarray(np.tile(np.asarray(b2, dtype=np.float32), 4)[:, None])
    in_maps = []
    for b in range(B):
        in_maps.append(
            {
                "qb": np.ascontiguousarray(qfb[b]),
                "wT": wT,
                "x": xf[b],
                "bpc": bpc,
                "g1c": g1c,
                "b1c": b1c,
                "g2r": g2r,
                "b2r": b2r,
            }
        )
    return in_maps


def _is_simple(bp, g1, b1, g2, b2):
    return (
        np.allclose(np.asarray(bp), 0)
        and np.allclose(np.asarray(g1), 1)
        and np.allclose(np.asarray(b1), 0)
        and np.allclose(np.asarray(g2), 1)
        and np.allclose(np.asarray(b2), 0)
    )


def _run(in_maps, simple=True, trace=False):
    from concourse.bass_utils import run_bass_kernel_spmd

    key = f"nc{int(simple)}"
    if key not in _CACHE:
        _CACHE[key] = _build_nc(simple)
    nc = _CACHE[key]
    res = run_bass_kernel_spmd(nc, in_maps, core_ids=list(range(B)), trace=trace)
    return res


def kernel(q, x, Wp, bp, g1, b1, g2, b2):
    simple = _is_simple(bp, g1, b1, g2, b2)
    _CACHE["simple"] = simple
    in_maps = _host_inputs(q, x, Wp, bp, g1, b1, g2, b2)
    res = _run(in_maps, simple=simple, trace=False)
    out = np.stack(
        [
            np.asarray(res.results[b]["out"]).astype(np.float32).reshape(CD, H, W)
            for b in range(B)
        ]
    )
    _CACHE["last_res"] = res
    return out


# revision 24
# speedup vs baseline: 1.2192x; 1.0326x over previous
"""Trainium2 Bass kernel for nn_HadaMard: fused proj + 2xLayerNorm + outer product.

Reference computation (per batch b, one NeuronCore per batch):
  qf = q[b].reshape(C1, N)           # [1024, 1024]
  proj = Wp @ qf (+ bp)              # [256, 1024]
  qn = LN_over_d(proj) * g1 + b1     # LN over the 256-channel dim
  xn = LN_over_e(x[b]) * g2 + b2     # LN over the 32-channel dim
  out[d*32+e, n] = qn[d, n] * xn[e, n]   # [8192, 1024]

Layout/strategy:
  - Output is transferred in bf16 (rel-err ~6e-3 << 2e-2 gate) and upcast on
    host: halves the dominant HBM write traffic.
  - Outer-product tiles are e-major: tile (md, e) holds out rows
    (128*md+p)*32+e for p in [0,128). The qn factor is the bf16 qn tile
    itself (no broadcast); the xn factor is one row broadcast to all 128
    partitions.
  - Row broadcasts go through a DRAM scratch roundtrip: A = xn is written
    once (ready early, x-side only), then each xbe tile is a stride-0
    partition-broadcast DMA read. These land on the DMA queues
    (sync/scalar/gpsimd) during the otherwise-idle window while the q-side
    LN chain runs, instead of loading the busy compute engines.
  - Elementwise products run on DVE (bf16 2x mode) and Pool, DMAs on
    sync/scalar/gpsimd, assigned by a static least-loaded balancer.
  - The q side is processed in two 512-column chunks (PSUM bank granularity);
    early e's run per-chunk products to start output DMA sooner, later e's
    run full-width products.
"""

import numpy as np

_CACHE = {}

B, C1, H, W = 8, 1024, 32, 32
C2 = 32
Cp = 256
N = H * W  # 1024
CD = Cp * C2  # 8192
EPS = 1e-5

ESPLIT = 4  # e < ESPLIT: per-chunk products; else full-width
NWU = 3  # PE warm-up matmuls
NFILL = 4  # PE filler matmuls between proj chunks


def _build_nc(simple):
    import concourse.bacc as bacc
    import concourse.bass as bass
    import concourse.mybir as mybir
    import concourse.tile as tile

    F32 = mybir.dt.float32
    F32R = mybir.dt.float32r
    BF16 = mybir.dt.bfloat16
    SQRT = mybir.ActivationFunctionType.Sqrt
    COPY = mybir.ActivationFunctionType.Copy
    MULT = mybir.AluOpType.mult
    ADD = mybir.AluOpType.add

    nc = bacc.Bacc(None, target_bir_lowering=False)

    q_d = nc.dram_tensor("qb", [C1, N], BF16, kind="ExternalInput")
    w_d = nc.dram_tensor("wT", [C1, Cp], BF16, kind="ExternalInput")
    x_d = nc.dram_tensor("x", [C2, N], F32, kind="ExternalInput")
    bp_d = nc.dram_tensor("bpc", [128, 2], F32, kind="ExternalInput")
    g1_d = nc.dram_tensor("g1c", [128, 2], F32, kind="ExternalInput")
    b1_d = nc.dram_tensor("b1c", [128, 2], F32, kind="ExternalInput")
    g2_d = nc.dram_tensor("g2r", [128, 1], F32, kind="ExternalInput")
    b2_d = nc.dram_tensor("b2r", [128, 1], F32, kind="ExternalInput")
    abuf_d = nc.dram_tensor("abuf", [128, 256], BF16, kind="Internal")  # packed: row 32a+e = A[e, 256a:]
    out_d = nc.dram_tensor("out", [CD, N], BF16, kind="ExternalOutput")

    # out view: row (md*128+p)*32+e  ->  [p, md, e, n]
    outv = out_d.rearrange("(md p e) n -> p md e n", md=2, p=128, e=C2)

    # ---- static least-loaded balancer (model-cost ns) ----
    clk = {"sync": 0.0, "scalar": 0.0, "gpsimd": 0.0, "vector": 0.0}

    def pick(cands, costs):
        e = min(cands, key=lambda x: clk[x])
        clk[e] += costs[e] if isinstance(costs, dict) else costs
        return e

    def charge(e, cost):
        clk[e] += cost

    DMA_ENGS = ["sync", "scalar", "gpsimd"]

    def dma_cost(bytes_per_part):
        return max(bytes_per_part * 0.3855, 500.0)

    def mul_costs(w):
        return {"vector": w * 1.0417 * 0.5 + 60.0, "gpsimd": w * 0.8333}

    CHUNKS = [(0, 512), (512, 1024)]

    with tile.TileContext(nc) as tc:
        with (
            tc.tile_pool(name="cst", bufs=1) as cst,
            tc.tile_pool(name="big", bufs=1) as big,
            tc.tile_pool(name="xbe", bufs=1) as xbp,
            tc.tile_pool(name="stg", bufs=7) as stg,
            tc.tile_pool(name="stc", bufs=4) as stc,
            tc.tile_pool(name="wrk", bufs=2) as wrk,
            tc.tile_pool(name="ps", bufs=4, space=bass.MemorySpace.PSUM) as ps,
            tc.tile_pool(name="wups", bufs=1, space=bass.MemorySpace.PSUM) as wups,
            tc.tile_pool(name="ps32", bufs=2, space=bass.MemorySpace.PSUM) as ps32,
        ):
            # ---- constants / warmup (t=0, no input deps) ----
            wu_l = cst.tile([128, 128], BF16, tag="wul")
            nc.vector.memset(wu_l[:], 0.5)
            wu_r = cst.tile([128, 512], BF16, tag="wur")
            nc.vector.memset(wu_r[:], 0.5)
            cq128 = cst.tile([128, 128], F32, tag="cq128")
            nc.vector.memset(cq128[:], 1.0 / Cp)
            cjx = cst.tile([128, 128], F32, tag="cjx")
            nc.vector.memset(cjx[:], 0.0)
            for a_ in range(4):
                nc.vector.memset(cjx[32 * a_ : 32 * (a_ + 1), 32 * a_ : 32 * (a_ + 1)], 1.0 / C2)
            eps32 = cst.tile([C2, 1], F32, tag="eps32")
            nc.vector.memset(eps32[:], EPS)
            eps128 = cst.tile([128, 1], F32, tag="eps128")
            nc.vector.memset(eps128[:], EPS)
            # preload the activation table early (ACT, off critical path)
            atl = cst.tile([C2, 1], F32, tag="atl")
            nc.scalar.copy(atl[:], eps32[:])
            nc.scalar.activation(atl[:], eps32[:], SQRT, bias=eps32[:])

            wu_ps = wups.tile([128, 512], F32, tag="wups")
            for i in range(NWU):
                nc.tensor.matmul(wu_ps[:], wu_l[:], wu_r[:], start=True, stop=True)

            def fillers(n):
                for _ in range(n):
                    nc.tensor.matmul(wu_ps[:], wu_l[:], wu_r[:], start=True, stop=True)

            # ---- input loads: x first (x-side chain is latency-critical) ----
            # packed layout: x4[32a+e, n'] = x[e, 256a+n']
            x_sb = cst.tile([128, 256], F32, tag="xs")
            charge("sync", dma_cost(1024))
            nc.sync.dma_start(
                x_sb[:], x_d.rearrange("e (a n) -> a e n", a=4)
            )
            q_sb = []
            for k in range(8):
                qt = big.tile([128, N], BF16, tag=f"q{k}")
                q_sb.append(qt)
                eng = "sync" if k < 4 else "scalar"
                charge(eng, dma_cost(2048))
                getattr(nc, eng).dma_start(qt[:], q_d[128 * k : 128 * (k + 1), :])
            # w merged: w2[j][p, 256c+d] = wT[128(4j+c)+p, d]
            w_sb = []
            for j in range(2):
                wt = big.tile([128, 4 * Cp], BF16, tag=f"w{j}")
                w_sb.append(wt)
                src = w_d[512 * j : 512 * (j + 1), :].rearrange("(c p) d -> p c d", c=4)
                dst = wt[:].rearrange("p (c d) -> p c d", c=4)
                charge("gpsimd", dma_cost(2048))
                nc.gpsimd.dma_start(dst, src)

            def wslice(k, md):
                j, c = divmod(k, 4)
                return w_sb[j][:, 256 * c + 128 * md : 256 * c + 128 * (md + 1)]

            def cload(dram, shape, tag):
                t = cst.tile(shape, F32, tag=tag)
                eng = pick(DMA_ENGS, dma_cost(shape[1] * 4))
                getattr(nc, eng).dma_start(t[:], dram[:])
                return t

            if not simple:
                bp_sb = cload(bp_d, [128, 2], "bp")
                g1_sb = cload(g1_d, [128, 2], "g1")
                b1_sb = cload(b1_d, [128, 2], "b1")
                g2_sb = cload(g2_d, [128, 1], "g2")
                b2_sb = cload(b2_d, [128, 1], "b2")

            # ---- x-side LN (packed [128,256]; independent of q) ----
            xsq = cst.tile([128, 256], F32, tag="xsq")
            nc.gpsimd.tensor_mul(xsq[:], x_sb[:], x_sb[:])
            charge("gpsimd", 256 * 0.8333)
            smx = ps32.tile([128, 256], F32, tag="s32")
            nc.tensor.matmul(
                smx[:], cjx[:].bitcast(F32R), x_sb[:].bitcast(F32R),
                start=True, stop=True,
            )
            sqx = ps32.tile([128, 256], F32, tag="s32")
            nc.tensor.matmul(
                sqx[:], cjx[:].bitcast(F32R), xsq[:].bitcast(F32R),
                start=True, stop=True,
            )
            mx = cst.tile([128, 256], F32, tag="mx")
            nc.scalar.copy(mx[:], smx[:])
            charge("scalar", 256 * 0.8333 + 370)
            mx2 = cst.tile([128, 256], F32, tag="mx2")
            nc.gpsimd.tensor_mul(mx2[:], mx[:], mx[:])
            charge("gpsimd", 256 * 0.8333)
            vx = cst.tile([128, 256], F32, tag="vx")
            nc.vector.tensor_sub(vx[:], sqx[:], mx2[:])
            charge("vector", 256 * 1.0417 + 125)
            sdx = cst.tile([128, 256], F32, tag="sdx")
            nc.scalar.activation(sdx[:], vx[:], SQRT, bias=eps128[:])
            charge("scalar", 256 * 0.8333 + 370)
            rsx = cst.tile([128, 256], F32, tag="rsx")
            nc.vector.reciprocal(rsx[:], sdx[:])
            charge("vector", 256 * 1.0417 + 60)
            xt = cst.tile([128, 256], F32, tag="xt")
            nc.gpsimd.tensor_sub(xt[:], x_sb[:], mx[:])
            charge("gpsimd", 256 * 0.8333)
            a_sb = cst.tile([128, 256], BF16, tag="a")
            if simple:
                nc.gpsimd.tensor_mul(a_sb[:], xt[:], rsx[:])
                charge("gpsimd", 256 * 0.8333)
            else:
                t3 = cst.tile([128, 256], F32, tag="t3")
                nc.gpsimd.tensor_mul(t3[:], xt[:], rsx[:])
                charge("gpsimd", 256 * 0.8333)
                nc.vector.tensor_scalar(
                    a_sb[:], t3[:], g2_sb[:, 0:1], b2_sb[:, 0:1], op0=MULT, op1=ADD
                )
                charge("vector", 256 * 1.0417 + 60)

            # A -> DRAM scratch (packed), then all 32 broadcast reads (fill
            # the DMA window while the q-side LN chain runs)
            charge("sync", dma_cost(512))
            nc.sync.dma_start(abuf_d[:], a_sb[:])
            abufv = abuf_d.rearrange("(a e) n -> e a n", a=4, e=C2)
            xbes = [None] * C2
            xbe_rr = [0]

            def emit_xbe(e, eng=None):
                if xbes[e] is not None:
                    return
                t = xbp.tile([128, N], BF16, tag=f"xbe{e}")
                src = abufv[e : e + 1, :, :].partition_broadcast(128)
                if eng is None:
                    eng = ["gpsimd", "sync", "gpsimd", "scalar"][xbe_rr[0] % 4]
                    xbe_rr[0] += 1
                charge(eng, dma_cost(N * 2))
                getattr(nc, eng).dma_start(
                    t[:].rearrange("p (a n) -> p a n", a=4), src
                )
                xbes[e] = t

            for e in range(6):
                emit_xbe(e, "sync")

            # ---- q-side: proj + LN -> cn (bf16, rstd folded in) ----
            cn = []
            for md in range(2):
                cnt = cst.tile([128, N], BF16, tag=f"cn{md}")
                cn.append(cnt)
            mb = cst.tile([128, N], F32, tag="mb")

            def qside_chunk(ci):
                c0, c1 = CHUNKS[ci]
                w_ = c1 - c0
                pj = []
                for md in range(2):
                    p_ = ps.tile([128, 512], F32, tag="ps")
                    for k in range(8):
                        nc.tensor.matmul(
                            p_[:, :w_], wslice(k, md), q_sb[k][:, c0:c1],
                            start=(k == 0), stop=(k == 7),
                        )
                    pj.append(p_)
                fillers(NFILL)

                projb, sq = [], []
                for md in range(2):
                    pb = wrk.tile([128, 512], F32, tag=f"pb{md}")
                    if simple:
                        nc.scalar.copy(pb[:, :w_], pj[md][:, :w_])
                    else:
                        nc.scalar.activation(
                            pb[:, :w_], pj[md][:, :w_], COPY,
                            bias=bp_sb[:, md : md + 1],
                        )
                    charge("scalar", w_ * 0.8333 + 370)
                    projb.append(pb)
                    s = wrk.tile([128, 512], F32, tag=f"sq{md}")
                    nc.gpsimd.tensor_mul(s[:, :w_], pb[:, :w_], pb[:, :w_])
                    charge("gpsimd", w_ * 0.8333)
                    sq.append(s)

                smq = ps.tile([128, 512], F32, tag="ps")
                for md in range(2):
                    nc.tensor.matmul(
                        smq[:, :w_], cq128[:].bitcast(F32R),
                        projb[md][:, :w_].bitcast(F32R),
                        start=(md == 0), stop=(md == 1),
                    )
                sqq = ps.tile([128, 512], F32, tag="ps")
                for md in range(2):
                    nc.tensor.matmul(
                        sqq[:, :w_], cq128[:].bitcast(F32R),
                        sq[md][:, :w_].bitcast(F32R),
                        start=(md == 0), stop=(md == 1),
                    )
                fillers(NFILL)

                nc.scalar.copy(mb[:, c0:c1], smq[:, :w_])
                charge("scalar", w_ * 0.8333 + 370)
                mb2 = wrk.tile([128, 512], F32, tag="mb2")
                nc.gpsimd.tensor_mul(mb2[:, :w_], mb[:, c0:c1], mb[:, c0:c1])
                charge("gpsimd", w_ * 0.8333)
                varq = wrk.tile([128, 512], F32, tag="varq")
                nc.vector.tensor_sub(varq[:, :w_], sqq[:, :w_], mb2[:, :w_])
                charge("vector", w_ * 1.0417 + 125)
                sdq = wrk.tile([128, 512], F32, tag="sdq")
                nc.scalar.activation(sdq[:, :w_], varq[:, :w_], SQRT, bias=eps128[:])
                charge("scalar", w_ * 0.8333 + 370)
                rstd = wrk.tile([128, 512], F32, tag="rstd")
                nc.vector.reciprocal(rstd[:, :w_], sdq[:, :w_])
                charge("vector", w_ * 1.0417 + 60)

                for md in range(2):
                    cs = wrk.tile([128, 512], F32, tag=f"cs{md}")
                    nc.gpsimd.tensor_sub(cs[:, :w_], projb[md][:, :w_], mb[:, c0:c1])
                    charge("gpsimd", w_ * 0.8333)
                    if simple:
                        nc.gpsimd.tensor_mul(cn[md][:, c0:c1], cs[:, :w_], rstd[:, :w_])
                        charge("gpsimd", w_ * 0.8333)
                    else:
                        c2_ = wrk.tile([128, 512], F32, tag=f"c2_{md}")
                        nc.gpsimd.tensor_mul(c2_[:, :w_], cs[:, :w_], rstd[:, :w_])
                        charge("gpsimd", w_ * 0.8333)
                        nc.vector.tensor_scalar(
                            cn[md][:, c0:c1], c2_[:, :w_],
                            g1_sb[:, md : md + 1], b1_sb[:, md : md + 1],
                            op0=MULT, op1=ADD,
                        )
                        charge("vector", w_ * 1.0417 + 60)

            POOL_E = {2, 5, 7}  # e % 8 in POOL_E -> Pool owns both products
            out_rr = [0]

            def emit_tile(e, c0, c1, force_eng=None, force_out=None):
                """products + staging + out DMA for tile column range [c0,c1)."""
                w_ = c1 - c0
                if w_ == N:
                    st = stg.tile([128, 2 * N], BF16, tag="st")
                else:
                    st = stc.tile([128, 2 * 512], BF16, tag="stc")
                sw = st.shape[1] // 2
                meng = force_eng or ("gpsimd" if (e % 5) == 2 else "vector")
                for md in range(2):
                    charge(meng, mul_costs(w_)[meng])
                    getattr(nc, meng).tensor_mul(
                        st[:, sw * md : sw * md + w_],
                        cn[md][:, c0:c1],
                        xbes[e][:, c0:c1],
                    )
                dst = outv[:, :, e, c0:c1]
                src = st[:].rearrange("p (md n) -> p md n", md=2)[:, :, :w_]
                if force_out is not None:
                    eng = force_out
                else:
                    eng = ["sync", "scalar", "gpsimd", "sync", "scalar", "sync", "scalar"][out_rr[0] % 7]
                    out_rr[0] += 1
                charge(eng, dma_cost(2 * w_ * 2))
                getattr(nc, eng).dma_start(dst, src)

            qside_chunk(0)
            # head tiles: DVE products, sync outs (other queues hold LN ops)
            for e in range(ESPLIT):
                emit_tile(e, 0, 512, force_eng="vector", force_out="sync")
            qside_chunk(1)
            for e in range(ESPLIT, C2):
                emit_xbe(min(e + 2, C2 - 1))
                emit_tile(e, 0, N)
            for e in range(ESPLIT):
                emit_tile(e, 512, N)

    nc.compile()
    return nc


def _host_inputs(q, x, Wp, bp, g1, b1, g2, b2):
    """Build the 8 per-core input maps."""
    import ml_dtypes

    qf = np.asarray(q, dtype=np.float32).reshape(B, C1, N)
    qfb = qf.astype(ml_dtypes.bfloat16)
    xf = np.ascontiguousarray(np.asarray(x, dtype=np.float32).reshape(B, C2, N))
    wT = np.ascontiguousarray(np.asarray(Wp, dtype=np.float32).T).astype(
        ml_dtypes.bfloat16
    )
    bpc = np.ascontiguousarray(np.asarray(bp, dtype=np.float32).reshape(2, 128).T)
    g1c = np.ascontiguousarray(np.asarray(g1, dtype=np.float32).reshape(2, 128).T)
    b1c = np.ascontiguousarray(np.asarray(b1, dtype=np.float32).reshape(2, 128).T)
    g2r = np.ascontiguousarray(np.tile(np.asarray(g2, dtype=np.float32), 4)[:, None])
    b2r = np.ascontiguousarray(np.tile(np.asarray(b2, dtype=np.float32), 4)[:, None])
    in_maps = []
    for b in range(B):
        in_maps.append(
            {
                "qb": np.ascontiguousarray(qfb[b]),
                "wT": wT,
                "x": xf[b],
                "bpc": bpc,
                "g1c": g1c,
                "b1c": b1c,
                "g2r": g2r,
                "b2r": b2r,
            }
        )
    return in_maps


def _is_simple(bp, g1, b1, g2, b2):
    return (
        np.allclose(np.asarray(bp), 0)
        and np.allclose(np.asarray(g1), 1)
        and np.allclose(np.asarray(b1), 0)
        and np.allclose(np.asarray(g2), 1)
        and np.allclose(np.asarray(b2), 0)
    )


def _run(in_maps, simple=True, trace=False):
    from concourse.bass_utils import run_bass_kernel_spmd

    key = f"nc{int(simple)}"
    if key not in _CACHE:
        _CACHE[key] = _build_nc(simple)
    nc = _CACHE[key]
    res = run_bass_kernel_spmd(nc, in_maps, core_ids=list(range(B)), trace=trace)
    return res


def kernel(q, x, Wp, bp, g1, b1, g2, b2):
    simple = _is_simple(bp, g1, b1, g2, b2)
    _CACHE["simple"] = simple
    in_maps = _host_inputs(q, x, Wp, bp, g1, b1, g2, b2)
    res = _run(in_maps, simple=simple, trace=False)
    out = np.stack(
        [
            np.asarray(res.results[b]["out"]).astype(np.float32).reshape(CD, H, W)
            for b in range(B)
        ]
    )
    _CACHE["last_res"] = res
    return out
